# revision 18
# baseline (speedup 1.0000x reference)
"""DGF layer kernel for Trainium2 (Bass/Tile), data-parallel over batch.

Reference computation (per sample, N=1024, D=256, H=8 heads):
    sq[n]   = sum_d x[n,d]^2
    dist2   = sq[n] + sq[m] - 2*x@x.T               (clamped >= 0)
    adj     = mean_h exp(-dist2 / (2*exp(ls_h)^2 + 1e-6))
    out     = LN(elu(adj @ x @ W.T + b) + x) * gamma + beta

Kernel strategy (one sample per NeuronCore, 8 cores):
    - Gram matmuls in bf16 (fast FWL weight loads); ACT computes
      E = exp(2a*G - a*sq[n]) with a per-partition bias, and a DVE bf16
      multiply by the host-precomputed row R[m] = (cnt/H)*exp(-a*sq[m])
      completes adj = (cnt/H)*exp(-a*dist2).  E overflows to inf on the
      diagonal (dist2==0 there) and R underflows, so the diagonal is patched
      to the exact value 1.0 (= sum_u cnt_u/H) with affine_select.
    - y = x @ W.T in float32r, then split into y_hi + y_lo (double-bf16);
      agg = adj @ y_hi + adj @ y_lo keeps fp32-level precision while using
      bf16 matmuls whose weight loads amortize 2x.
    - elu(z) + x = max(z,0) + min(exp(z),1) + (x - 1); x-1 prepared on host,
      both adds fused into two scalar_tensor_tensor ops.
    - LayerNorm: bn_stats/bn_aggr; rstd via batched quake-seed Newton rsqrt
      on DVE (ACT Sqrt lives in a different act-table set than Exp; mixing
      would cost a 1.3us table reload per switch); the normalize itself runs
      on ACT as Identity(v*rstd + (-mean*rstd)).
    - A short burst of dummy matmuls at kernel start warms the PE HAM clock
      (1.2 -> 2.4 GHz) while the input DMAs are still in flight.
"""

from contextlib import ExitStack

import numpy as np

B, N, D, H = 8, 1024, 256, 8
LN_EPS = 1e-5
P = 128
NT = N // P  # 8 row/col blocks
DC = D // P  # 2 contraction chunks

_PROGRAM_CACHE = {}


def _build_program(n_alpha, two_alphas, need_b, need_gamma, need_beta):
    import concourse.bass as bass
    import concourse.tile as tile
    from concourse import bacc, mybir

    f32 = mybir.dt.float32
    bf16 = mybir.dt.bfloat16
    i32 = mybir.dt.int32
    AF = mybir.ActivationFunctionType
    OP = mybir.AluOpType

    nc = bacc.Bacc("TRN2", target_bir_lowering=False, debug=False, enable_asserts=False)

    xhi_d = nc.dram_tensor("xhi", [D, N], bf16, kind="ExternalInput").ap()
    xlo_d = nc.dram_tensor("xlo", [D, N], bf16, kind="ExternalInput").ap()
    xm1_d = nc.dram_tensor("xm1", [N, D], f32, kind="ExternalInput").ap()
    wcat_d = nc.dram_tensor("wcat", [D, 2 * D], bf16, kind="ExternalInput").ap()
    nasq_d = nc.dram_tensor("nasq", [n_alpha, N], f32, kind="ExternalInput").ap()
    rbf_d = nc.dram_tensor("rbf", [n_alpha, N], bf16, kind="ExternalInput").ap()
    brow_d = grow_d = berow_d = None
    if need_b:
        brow_d = nc.dram_tensor("brow", [D], f32, kind="ExternalInput").ap()
    if need_gamma:
        grow_d = nc.dram_tensor("grow", [D], f32, kind="ExternalInput").ap()
    if need_beta:
        berow_d = nc.dram_tensor("berow", [D], f32, kind="ExternalInput").ap()
    out_d = nc.dram_tensor("out", [N, D], f32, kind="ExternalOutput").ap()

    def bcast_ap(src):
        return bass.AP(tensor=src.tensor, offset=src.offset,
                       ap=[[0, P]] + list(src.ap))

    with tile.TileContext(nc) as tc, ExitStack() as ctx:
        singles = ctx.enter_context(tc.tile_pool(name="singles", bufs=1))
        work = ctx.enter_context(tc.tile_pool(name="work", bufs=4))
        stats = ctx.enter_context(tc.tile_pool(name="stats", bufs=4))

        # ---- persistent SBUF tensors ----
        xhi_sb = singles.tile([P, DC, N], bf16)
        xlo_sb = singles.tile([P, DC, N], bf16)
        xm1_sb = singles.tile([P, NT, D], f32)
        wcat_sb = singles.tile([P, DC, 2 * D], bf16)
        yhi_sb = singles.tile([P, NT, D], bf16)
        ylo_sb = singles.tile([P, NT, D], bf16)
        y32_sb = singles.tile([P, NT, D], f32)
        adj_sb = singles.tile([P, NT, N], bf16)
        v_sb = singles.tile([P, NT, D], f32)
        out_sb = singles.tile([P, NT, D], f32)
        nasq_sb = singles.tile([P, n_alpha, NT], f32)
        rbf_sb = singles.tile([P, n_alpha, N], bf16)
        mv_sb = singles.tile([P, NT, 2], f32)
        rstd_sb = singles.tile([P, NT], f32)
        nmr_sb = singles.tile([P, NT], f32)
        magic_sb = singles.tile([P, NT], i32)
        warm_sb = singles.tile([P, 512], bf16)

        i8 = mybir.dt.int8
        eyem_sb = singles.tile([P, P], i8)
        ones_sb = singles.tile([P, P], bf16)
        nc.vector.memset(magic_sb[:], 0x5F3759DF)
        nc.gpsimd.memset(warm_sb[:], 0.5)
        nc.gpsimd.memset(ones_sb[:], 1.0)
        nc.gpsimd.memset(eyem_sb[:], 1)
        nc.gpsimd.affine_select(
            out=eyem_sb[:], in_=eyem_sb[:], compare_op=OP.is_equal,
            fill=0, base=0, channel_multiplier=1, pattern=[[-1, P]],
        )

        # ---- PE warmup while input DMAs land (HAM 1.2 -> 2.4 GHz) ----
        warm_pool = tc.tile_pool(name="warm_psum", bufs=1, space="PSUM")
        wp = warm_pool.__enter__()
        pwarm = wp.tile([P, 512], f32)
        for _ in range(8):
            nc.tensor.matmul(pwarm[:], warm_sb[:, 0:P], warm_sb[:],
                             start=True, stop=True)
        warm_pool.__exit__(None, None, None)

        # ---- input DMAs, urgency-ordered across the 3 dispatch engines ----
        def chunk(ap2d):
            return [ap2d[c * P:(c + 1) * P, :]
                    .rearrange("(o p) n -> p o n", p=P) for c in range(DC)]

        xhi_c = chunk(xhi_d)
        xlo_c = chunk(xlo_d)
        nc.sync.dma_start(out=xhi_sb[:, 0:1, :], in_=xhi_c[0])
        nc.gpsimd.dma_start(out=xhi_sb[:, 1:2, :], in_=xhi_c[1])
        nc.scalar.dma_start(out=wcat_sb[:], in_=wcat_d.rearrange("(c p) e -> p c e", p=P))
        nc.sync.dma_start(out=nasq_sb[:], in_=nasq_d.rearrange("u (t p) -> p u t", p=P))
        for u in range(n_alpha):
            nc.scalar.dma_start(out=rbf_sb[:, u, :], in_=bcast_ap(rbf_d[u]))
        nc.scalar.dma_start(out=xlo_sb[:, 0:1, :], in_=xlo_c[0])
        nc.gpsimd.dma_start(out=xlo_sb[:, 1:2, :], in_=xlo_c[1])
        xm1_v = xm1_d.rearrange("(t p) d -> p t d", p=P)
        nc.sync.dma_start(out=xm1_sb[:, 0:4, :], in_=xm1_v[:, 0:4, :])
        nc.gpsimd.dma_start(out=xm1_sb[:, 4:8, :], in_=xm1_v[:, 4:8, :])
        b_bc = g_bc = be_bc = None
        if need_b:
            b_bc = singles.tile([P, D], f32)
            nc.scalar.dma_start(out=b_bc[:], in_=bcast_ap(brow_d))
        if need_gamma:
            g_bc = singles.tile([P, D], f32)
            nc.scalar.dma_start(out=g_bc[:], in_=bcast_ap(grow_d))
        if need_beta:
            be_bc = singles.tile([P, D], f32)
            nc.scalar.dma_start(out=be_bc[:], in_=bcast_ap(berow_d))

        # ---- adjacency rows ----
        g_pool = tc.tile_pool(name="g_psum", bufs=3, space="PSUM")
        gp = g_pool.__enter__()
        for a in range(NT):
            pg = gp.tile([P, N], f32)
            for h in range(2):
                sl = slice(h * 512, (h + 1) * 512)
                for c in range(DC):
                    nc.tensor.matmul(
                        pg[:, sl],
                        xhi_sb[:, c, a * P:(a + 1) * P],
                        xhi_sb[:, c, sl],
                        start=(c == 0),
                        stop=(c == DC - 1),
                    )
            for u in range(n_alpha):
                et = work.tile([P, N], bf16, tag="et")
                nc.scalar.activation(
                    et[:], pg[:], AF.Exp,
                    bias=nasq_sb[:, u, a:a + 1], scale=two_alphas[u],
                )
                if u == 0:
                    nc.vector.tensor_tensor(
                        adj_sb[:, a, :], et[:], rbf_sb[:, 0, :], OP.mult
                    )
                else:
                    tmp = work.tile([P, N], bf16, tag="tmpu")
                    nc.vector.tensor_tensor(tmp[:], et[:], rbf_sb[:, u, :], OP.mult)
                    nc.vector.tensor_tensor(
                        adj_sb[:, a, :], adj_sb[:, a, :], tmp[:], OP.add
                    )
            # dist2(n,n) == 0 -> adjacency diagonal is exactly sum_u cnt_u/H = 1
            nc.vector.copy_predicated(
                out=adj_sb[:, a, a * P:(a + 1) * P], mask=eyem_sb[:], data=ones_sb[:]
            )

        # ---- y = x @ W.T (double-bf16: psum = [x@w_hi | x@w_lo]) ----
        y_pool = tc.tile_pool(name="y_psum", bufs=2, space="PSUM")
        yp = y_pool.__enter__()
        for j in range(NT):
            py = yp.tile([P, D], f32)
            k = 0
            for xs in (xhi_sb, xlo_sb):
                for c in range(DC):
                    for wh in range(2):
                        nc.tensor.matmul(
                            py[:],
                            xs[:, c, j * P:(j + 1) * P],
                            wcat_sb[:, c, wh * D:(wh + 1) * D],
                            start=(k == 0),
                            stop=(k == 7),
                        )
                        k += 1
            nc.vector.tensor_copy(y32_sb[:, j, :], py[:])
            nc.gpsimd.tensor_copy(yhi_sb[:, j, :], y32_sb[:, j, :])
            nc.gpsimd.tensor_sub(ylo_sb[:, j, :], y32_sb[:, j, :], yhi_sb[:, j, :])
        y_pool.__exit__(None, None, None)
        g_pool.__exit__(None, None, None)

        # ---- agg = adj @ (y_hi + y_lo), elu, +x, stats, LN ----
        a_pool = tc.tile_pool(name="a_psum", bufs=8, space="PSUM")
        ap_ = a_pool.__enter__()

        def newton_batch(bi):
            bs = slice(bi * 4, bi * 4 + 4)
            wv = stats.tile([P, 4], f32, tag="wv")
            nc.vector.tensor_scalar_add(wv[:], mv_sb[:, bs, 1], LN_EPS)
            sh = stats.tile([P, 4], i32, tag="sh")
            nc.vector.tensor_scalar(
                sh[:], wv[:].bitcast(i32), 1, None, OP.arith_shift_right
            )
            nc.vector.tensor_tensor(
                rstd_sb[:, bs].bitcast(i32), magic_sb[:, bs], sh[:], OP.subtract
            )
            na = stats.tile([P, 4], f32, tag="na")
            nb = stats.tile([P, 4], f32, tag="nb")
            for _ in range(2):
                nc.vector.tensor_tensor(na[:], rstd_sb[:, bs], rstd_sb[:, bs], OP.mult)
                nc.vector.scalar_tensor_tensor(nb[:], na[:], -0.5, wv[:], OP.mult, OP.mult)
                nc.vector.scalar_tensor_tensor(
                    rstd_sb[:, bs], nb[:], 1.5, rstd_sb[:, bs], OP.add, OP.mult
                )
            nc.vector.scalar_tensor_tensor(
                nmr_sb[:, bs], mv_sb[:, bs, 0], -1.0, rstd_sb[:, bs], OP.mult, OP.mult
            )

        out_view = out_d.rearrange("(t p) d -> p t d", p=P)
        for i in range(NT):
            pa = ap_.tile([P, D], f32)
            for j in range(NT):
                nc.tensor.matmul(
                    pa[:],
                    adj_sb[:, j, i * P:(i + 1) * P],
                    yhi_sb[:, j, :],
                    start=(j == 0),
                    stop=False,
                )
                nc.tensor.matmul(
                    pa[:],
                    adj_sb[:, j, i * P:(i + 1) * P],
                    ylo_sb[:, j, :],
                    start=False,
                    stop=(j == NT - 1),
                )
            if need_b:
                zsb = work.tile([P, D], f32, tag="zsb")
                nc.vector.tensor_tensor(zsb[:], pa[:], b_bc[:], OP.add)
                zin = zsb
            else:
                zin = pa
            e = work.tile([P, D], f32, tag="e")
            nc.scalar.activation(e[:], zin[:], AF.Exp)
            rx = work.tile([P, D], f32, tag="rx")
            nc.vector.scalar_tensor_tensor(
                rx[:], zin[:], 0.0, xm1_sb[:, i, :], OP.max, OP.add
            )
            nc.vector.scalar_tensor_tensor(
                v_sb[:, i, :], e[:], 1.0, rx[:], OP.min, OP.add
            )
            st = stats.tile([P, 6], f32, tag="st")
            nc.vector.bn_stats(st[:], v_sb[:, i, :])
            nc.vector.bn_aggr(mv_sb[:, i, :], st[:])
            if i == 3:
                newton_batch(0)
            if i == 7:
                newton_batch(1)

        # ---- normalize (ACT Identity) + optional affine + store ----
        for i in range(NT):
            nc.scalar.activation(
                out_sb[:, i, :], v_sb[:, i, :], AF.Identity,
                bias=nmr_sb[:, i:i + 1], scale=rstd_sb[:, i:i + 1],
            )
            if need_gamma:
                nc.vector.tensor_mul(out_sb[:, i, :], out_sb[:, i, :], g_bc[:])
            if need_beta:
                nc.vector.tensor_add(out_sb[:, i, :], out_sb[:, i, :], be_bc[:])
            if i % 2 == 0:
                nc.sync.dma_start(out=out_view[:, i, :], in_=out_sb[:, i, :])
            else:
                nc.gpsimd.dma_start(out=out_view[:, i, :], in_=out_sb[:, i, :])

        a_pool.__exit__(None, None, None)

    nc.compile()
    return nc


def _prepare_core_inputs(x_k, alphas, weights, W_T, b_proj, ln_gamma, ln_beta,
                         need_b, need_gamma, need_beta):
    import ml_dtypes

    bf = ml_dtypes.bfloat16
    xf = np.ascontiguousarray(x_k, dtype=np.float32)
    sq = np.sum(xf * xf, axis=-1, dtype=np.float32)
    xT = np.ascontiguousarray(xf.T)
    xhi = xT.astype(bf)
    xlo = (xT - xhi.astype(np.float32)).astype(bf)
    whi = W_T.astype(bf)
    wlo = (W_T - whi.astype(np.float32)).astype(bf)
    m = {
        "xhi": xhi,
        "xlo": xlo,
        "xm1": (xf - np.float32(1.0)),
        "wcat": np.concatenate([whi, wlo], axis=1),
        "nasq": np.stack([(-a) * sq for a in alphas]).astype(np.float32),
        "rbf": np.stack(
            [w * np.exp((-a) * sq.astype(np.float64)) for a, w in zip(alphas, weights)]
        ).astype(bf),
    }
    if need_b:
        m["brow"] = b_proj
    if need_gamma:
        m["grow"] = ln_gamma
    if need_beta:
        m["berow"] = ln_beta
    return m


def _specialize(inputs):
    x = np.asarray(inputs["x"], dtype=np.float32)
    log_sigmas = np.asarray(inputs["log_sigmas"], dtype=np.float32)
    W_proj = np.asarray(inputs["W_proj"], dtype=np.float32)
    b_proj = np.ascontiguousarray(np.asarray(inputs["b_proj"], dtype=np.float32))
    ln_gamma = np.ascontiguousarray(np.asarray(inputs["ln_gamma"], dtype=np.float32))
    ln_beta = np.ascontiguousarray(np.asarray(inputs["ln_beta"], dtype=np.float32))

    sigmas = np.exp(log_sigmas)
    denoms = (np.float32(2.0) * sigmas * sigmas + np.float32(1e-6)).astype(np.float32)
    uniq, counts = np.unique(denoms, return_counts=True)
    alphas = (np.float32(1.0) / uniq).astype(np.float32)
    weights = counts.astype(np.float32) / np.float32(H)
    two_alphas = tuple(float(2.0 * a) for a in alphas)

    need_b = bool(np.any(b_proj != 0))
    need_gamma = not bool(np.all(ln_gamma == 1))
    need_beta = bool(np.any(ln_beta != 0))
    return (x, W_proj, b_proj, ln_gamma, ln_beta, alphas, weights, two_alphas,
            need_b, need_gamma, need_beta)


def kernel(**inputs):
    from concourse import bass_utils

    (x, W_proj, b_proj, ln_gamma, ln_beta, alphas, weights, two_alphas,
     need_b, need_gamma, need_beta) = _specialize(inputs)

    assert x.shape == (B, N, D), x.shape

    key = (len(alphas), two_alphas, tuple(float(v) for v in weights),
           need_b, need_gamma, need_beta)
    if key not in _PROGRAM_CACHE:
        _PROGRAM_CACHE[key] = _build_program(
            len(alphas), two_alphas, need_b, need_gamma, need_beta
        )
    nc = _PROGRAM_CACHE[key]

    W_T = np.ascontiguousarray(W_proj.T)
    in_maps = [
        _prepare_core_inputs(x[k], alphas, weights, W_T, b_proj, ln_gamma, ln_beta,
                             need_b, need_gamma, need_beta)
        for k in range(B)
    ]

    res = bass_utils.run_bass_kernel_spmd(nc, in_maps, core_ids=list(range(B)))
    out = np.stack([res.results[k]["out"] for k in range(B)])
    return out.astype(np.float32)


if __name__ == "__main__":
    import reference as R

    inp = R.setup_inputs()
    got = kernel(**{k: np.asarray(v) for k, v in inp.items()})
    print("out shape", got.shape, got.dtype)


# revision 19
# speedup vs baseline: 1.0104x; 1.0104x over previous
"""DGF layer kernel for Trainium2 (Bass/Tile), data-parallel over batch.

Reference computation (per sample, N=1024, D=256, H=8 heads):
    sq[n]   = sum_d x[n,d]^2
    dist2   = sq[n] + sq[m] - 2*x@x.T               (clamped >= 0)
    adj     = mean_h exp(-dist2 / (2*exp(ls_h)^2 + 1e-6))
    out     = LN(elu(adj @ x @ W.T + b) + x) * gamma + beta

Kernel strategy (one sample per NeuronCore, 8 cores):
    - Gram matmuls in bf16 (fast FWL weight loads); ACT computes
      E = exp(2a*G - a*sq[n]) with a per-partition bias, and a DVE bf16
      multiply by the host-precomputed row R[m] = (cnt/H)*exp(-a*sq[m])
      completes adj = (cnt/H)*exp(-a*dist2).  E overflows to inf on the
      diagonal (dist2==0 there) and R underflows, so the diagonal is patched
      to the exact value 1.0 (= sum_u cnt_u/H) with affine_select.
    - y = x @ W.T in float32r, then split into y_hi + y_lo (double-bf16);
      agg = adj @ y_hi + adj @ y_lo keeps fp32-level precision while using
      bf16 matmuls whose weight loads amortize 2x.
    - elu(z) + x = max(z,0) + min(exp(z),1) + (x - 1); x-1 prepared on host,
      both adds fused into two scalar_tensor_tensor ops.
    - LayerNorm: bn_stats/bn_aggr; rstd via batched quake-seed Newton rsqrt
      on DVE (ACT Sqrt lives in a different act-table set than Exp; mixing
      would cost a 1.3us table reload per switch); the normalize itself runs
      on ACT as Identity(v*rstd + (-mean*rstd)).
    - A short burst of dummy matmuls at kernel start warms the PE HAM clock
      (1.2 -> 2.4 GHz) while the input DMAs are still in flight.
"""

from contextlib import ExitStack

import numpy as np

B, N, D, H = 8, 1024, 256, 8
LN_EPS = 1e-5
P = 128
NT = N // P  # 8 row/col blocks
DC = D // P  # 2 contraction chunks

_PROGRAM_CACHE = {}


def _build_program(n_alpha, two_alphas, need_b, need_gamma, need_beta):
    import concourse.bass as bass
    import concourse.tile as tile
    from concourse import bacc, mybir

    f32 = mybir.dt.float32
    bf16 = mybir.dt.bfloat16
    i32 = mybir.dt.int32
    AF = mybir.ActivationFunctionType
    OP = mybir.AluOpType

    nc = bacc.Bacc("TRN2", target_bir_lowering=False, debug=False, enable_asserts=False)

    xhi_d = nc.dram_tensor("xhi", [D, N], bf16, kind="ExternalInput").ap()
    xlo_d = nc.dram_tensor("xlo", [D, N], bf16, kind="ExternalInput").ap()
    xm1_d = nc.dram_tensor("xm1", [N, D], f32, kind="ExternalInput").ap()
    wcat_d = nc.dram_tensor("wcat", [D, 2 * D], bf16, kind="ExternalInput").ap()
    nasq_d = nc.dram_tensor("nasq", [n_alpha, N], f32, kind="ExternalInput").ap()
    rbf_d = nc.dram_tensor("rbf", [n_alpha, N], bf16, kind="ExternalInput").ap()
    brow_d = grow_d = berow_d = None
    if need_b:
        brow_d = nc.dram_tensor("brow", [D], f32, kind="ExternalInput").ap()
    if need_gamma:
        grow_d = nc.dram_tensor("grow", [D], f32, kind="ExternalInput").ap()
    if need_beta:
        berow_d = nc.dram_tensor("berow", [D], f32, kind="ExternalInput").ap()
    out_d = nc.dram_tensor("out", [N, D], f32, kind="ExternalOutput").ap()

    def bcast_ap(src):
        return bass.AP(tensor=src.tensor, offset=src.offset,
                       ap=[[0, P]] + list(src.ap))

    with tile.TileContext(nc) as tc, ExitStack() as ctx:
        singles = ctx.enter_context(tc.tile_pool(name="singles", bufs=1))
        work = ctx.enter_context(tc.tile_pool(name="work", bufs=4))
        stats = ctx.enter_context(tc.tile_pool(name="stats", bufs=4))

        # ---- persistent SBUF tensors ----
        xhi_sb = singles.tile([P, DC, N], bf16)
        xlo_sb = singles.tile([P, DC, N], bf16)
        xm1_sb = singles.tile([P, NT, D], f32)
        wcat_sb = singles.tile([P, DC, 2 * D], bf16)
        yhi_sb = singles.tile([P, NT, D], bf16)
        ylo_sb = singles.tile([P, NT, D], bf16)
        y32_sb = singles.tile([P, NT, D], f32)
        adj_sb = singles.tile([P, NT, N], bf16)
        v_sb = singles.tile([P, NT, D], f32)
        out_sb = singles.tile([P, NT, D], f32)
        nasq_sb = singles.tile([P, n_alpha, NT], f32)
        rbf_sb = singles.tile([P, n_alpha, N], bf16)
        mv_sb = singles.tile([P, NT, 2], f32)
        rstd_sb = singles.tile([P, NT], f32)
        nmr_sb = singles.tile([P, NT], f32)
        magic_sb = singles.tile([P, NT], i32)
        warm_sb = singles.tile([P, 512], bf16)

        i8 = mybir.dt.int8
        eyem_sb = singles.tile([P, P], i8)
        ones_sb = singles.tile([P, P], bf16)
        nc.vector.memset(magic_sb[:], 0x5F3759DF)
        nc.gpsimd.memset(warm_sb[:], 0.5)
        nc.gpsimd.memset(ones_sb[:], 1.0)
        nc.gpsimd.memset(eyem_sb[:], 1)
        nc.gpsimd.affine_select(
            out=eyem_sb[:], in_=eyem_sb[:], compare_op=OP.is_equal,
            fill=0, base=0, channel_multiplier=1, pattern=[[-1, P]],
        )

        # ---- PE warmup while input DMAs land (HAM 1.2 -> 2.4 GHz) ----
        warm_pool = tc.tile_pool(name="warm_psum", bufs=1, space="PSUM")
        wp = warm_pool.__enter__()
        pwarm = wp.tile([P, 512], f32)
        for _ in range(8):
            nc.tensor.matmul(pwarm[:], warm_sb[:, 0:P], warm_sb[:],
                             start=True, stop=True)
        warm_pool.__exit__(None, None, None)

        # ---- input DMAs, urgency-ordered across the 3 dispatch engines ----
        def chunk(ap2d):
            return [ap2d[c * P:(c + 1) * P, :]
                    .rearrange("(o p) n -> p o n", p=P) for c in range(DC)]

        xhi_c = chunk(xhi_d)
        xlo_c = chunk(xlo_d)
        nc.sync.dma_start(out=xhi_sb[:, 0:1, :], in_=xhi_c[0])
        nc.gpsimd.dma_start(out=xhi_sb[:, 1:2, :], in_=xhi_c[1])
        nc.scalar.dma_start(out=wcat_sb[:], in_=wcat_d.rearrange("(c p) e -> p c e", p=P))
        nc.sync.dma_start(out=nasq_sb[:], in_=nasq_d.rearrange("u (t p) -> p u t", p=P))
        for u in range(n_alpha):
            nc.scalar.dma_start(out=rbf_sb[:, u, :], in_=bcast_ap(rbf_d[u]))
        nc.scalar.dma_start(out=xlo_sb[:, 0:1, :], in_=xlo_c[0])
        nc.gpsimd.dma_start(out=xlo_sb[:, 1:2, :], in_=xlo_c[1])
        xm1_v = xm1_d.rearrange("(t p) d -> p t d", p=P)
        nc.sync.dma_start(out=xm1_sb[:, 0:4, :], in_=xm1_v[:, 0:4, :])
        nc.gpsimd.dma_start(out=xm1_sb[:, 4:8, :], in_=xm1_v[:, 4:8, :])
        b_bc = g_bc = be_bc = None
        if need_b:
            b_bc = singles.tile([P, D], f32)
            nc.scalar.dma_start(out=b_bc[:], in_=bcast_ap(brow_d))
        if need_gamma:
            g_bc = singles.tile([P, D], f32)
            nc.scalar.dma_start(out=g_bc[:], in_=bcast_ap(grow_d))
        if need_beta:
            be_bc = singles.tile([P, D], f32)
            nc.scalar.dma_start(out=be_bc[:], in_=bcast_ap(berow_d))

        # ---- adjacency rows ----
        g_pool = tc.tile_pool(name="g_psum", bufs=3, space="PSUM")
        gp = g_pool.__enter__()
        for a in range(NT):
            pg = gp.tile([P, N], f32)
            for c in range(DC):
                for h in range(2):
                    sl = slice(h * 512, (h + 1) * 512)
                    nc.tensor.matmul(
                        pg[:, sl],
                        xhi_sb[:, c, a * P:(a + 1) * P],
                        xhi_sb[:, c, sl],
                        start=(c == 0),
                        stop=(c == DC - 1),
                    )
            for u in range(n_alpha):
                et = work.tile([P, N], bf16, tag="et")
                nc.scalar.activation(
                    et[:], pg[:], AF.Exp,
                    bias=nasq_sb[:, u, a:a + 1], scale=two_alphas[u],
                )
                if u == 0:
                    nc.vector.tensor_tensor(
                        adj_sb[:, a, :], et[:], rbf_sb[:, 0, :], OP.mult
                    )
                else:
                    tmp = work.tile([P, N], bf16, tag="tmpu")
                    nc.vector.tensor_tensor(tmp[:], et[:], rbf_sb[:, u, :], OP.mult)
                    nc.vector.tensor_tensor(
                        adj_sb[:, a, :], adj_sb[:, a, :], tmp[:], OP.add
                    )
            # dist2(n,n) == 0 -> adjacency diagonal is exactly sum_u cnt_u/H = 1
            nc.vector.copy_predicated(
                out=adj_sb[:, a, a * P:(a + 1) * P], mask=eyem_sb[:], data=ones_sb[:]
            )

        # ---- y = x @ W.T (double-bf16: psum = [x@w_hi | x@w_lo]) ----
        y_pool = tc.tile_pool(name="y_psum", bufs=2, space="PSUM")
        yp = y_pool.__enter__()
        for j in range(NT):
            py = yp.tile([P, D], f32)
            terms = [(xhi_sb, 0), (xhi_sb, 1), (xlo_sb, 0)]
            k = 0
            nmm = 2 * len(terms)
            for xs, wh_only in [(xhi_sb, None), (xlo_sb, 0)]:
                for c in range(DC):
                    whs = (0, 1) if wh_only is None else (wh_only,)
                    for wh in whs:
                        nc.tensor.matmul(
                            py[:],
                            xs[:, c, j * P:(j + 1) * P],
                            wcat_sb[:, c, wh * D:(wh + 1) * D],
                            start=(k == 0),
                            stop=(k == 5),
                        )
                        k += 1
            nc.vector.tensor_copy(y32_sb[:, j, :], py[:])
            nc.gpsimd.tensor_copy(yhi_sb[:, j, :], y32_sb[:, j, :])
            nc.gpsimd.tensor_sub(ylo_sb[:, j, :], y32_sb[:, j, :], yhi_sb[:, j, :])
        y_pool.__exit__(None, None, None)
        g_pool.__exit__(None, None, None)

        # ---- agg = adj @ (y_hi + y_lo), elu, +x, stats, LN ----
        a_pool = tc.tile_pool(name="a_psum", bufs=8, space="PSUM")
        ap_ = a_pool.__enter__()

        def newton_batch(bi):
            bs = slice(bi * 4, bi * 4 + 4)
            wv = stats.tile([P, 4], f32, tag="wv")
            nc.vector.tensor_scalar_add(wv[:], mv_sb[:, bs, 1], LN_EPS)
            sh = stats.tile([P, 4], i32, tag="sh")
            nc.vector.tensor_scalar(
                sh[:], wv[:].bitcast(i32), 1, None, OP.arith_shift_right
            )
            nc.vector.tensor_tensor(
                rstd_sb[:, bs].bitcast(i32), magic_sb[:, bs], sh[:], OP.subtract
            )
            na = stats.tile([P, 4], f32, tag="na")
            nb = stats.tile([P, 4], f32, tag="nb")
            for _ in range(2):
                nc.vector.tensor_tensor(na[:], rstd_sb[:, bs], rstd_sb[:, bs], OP.mult)
                nc.vector.scalar_tensor_tensor(nb[:], na[:], -0.5, wv[:], OP.mult, OP.mult)
                nc.vector.scalar_tensor_tensor(
                    rstd_sb[:, bs], nb[:], 1.5, rstd_sb[:, bs], OP.add, OP.mult
                )
            nc.vector.scalar_tensor_tensor(
                nmr_sb[:, bs], mv_sb[:, bs, 0], -1.0, rstd_sb[:, bs], OP.mult, OP.mult
            )

        out_view = out_d.rearrange("(t p) d -> p t d", p=P)
        for i in range(NT):
            pa = ap_.tile([P, D], f32)
            for j in range(NT):
                nc.tensor.matmul(
                    pa[:],
                    adj_sb[:, j, i * P:(i + 1) * P],
                    yhi_sb[:, j, :],
                    start=(j == 0),
                    stop=False,
                )
                nc.tensor.matmul(
                    pa[:],
                    adj_sb[:, j, i * P:(i + 1) * P],
                    ylo_sb[:, j, :],
                    start=False,
                    stop=(j == NT - 1),
                )
            if need_b:
                zsb = work.tile([P, D], f32, tag="zsb")
                nc.vector.tensor_tensor(zsb[:], pa[:], b_bc[:], OP.add)
                zin = zsb
            else:
                zin = pa
            e = work.tile([P, D], f32, tag="e")
            nc.scalar.activation(e[:], zin[:], AF.Exp)
            rx = work.tile([P, D], f32, tag="rx")
            nc.vector.scalar_tensor_tensor(
                rx[:], zin[:], 0.0, xm1_sb[:, i, :], OP.max, OP.add
            )
            nc.vector.scalar_tensor_tensor(
                v_sb[:, i, :], e[:], 1.0, rx[:], OP.min, OP.add
            )
            st = stats.tile([P, 6], f32, tag="st")
            nc.vector.bn_stats(st[:], v_sb[:, i, :])
            nc.vector.bn_aggr(mv_sb[:, i, :], st[:])
            if i == 3:
                newton_batch(0)
            if i == 7:
                newton_batch(1)

        # ---- normalize (ACT Identity) + optional affine + store ----
        for i in range(NT):
            nc.scalar.activation(
                out_sb[:, i, :], v_sb[:, i, :], AF.Identity,
                bias=nmr_sb[:, i:i + 1], scale=rstd_sb[:, i:i + 1],
            )
            if need_gamma:
                nc.vector.tensor_mul(out_sb[:, i, :], out_sb[:, i, :], g_bc[:])
            if need_beta:
                nc.vector.tensor_add(out_sb[:, i, :], out_sb[:, i, :], be_bc[:])
            if i % 2 == 0:
                nc.sync.dma_start(out=out_view[:, i, :], in_=out_sb[:, i, :])
            else:
                nc.gpsimd.dma_start(out=out_view[:, i, :], in_=out_sb[:, i, :])

        a_pool.__exit__(None, None, None)

    nc.compile()
    return nc


def _prepare_core_inputs(x_k, alphas, weights, W_T, b_proj, ln_gamma, ln_beta,
                         need_b, need_gamma, need_beta):
    import ml_dtypes

    bf = ml_dtypes.bfloat16
    xf = np.ascontiguousarray(x_k, dtype=np.float32)
    sq = np.sum(xf * xf, axis=-1, dtype=np.float32)
    xT = np.ascontiguousarray(xf.T)
    xhi = xT.astype(bf)
    xlo = (xT - xhi.astype(np.float32)).astype(bf)
    whi = W_T.astype(bf)
    wlo = (W_T - whi.astype(np.float32)).astype(bf)
    m = {
        "xhi": xhi,
        "xlo": xlo,
        "xm1": (xf - np.float32(1.0)),
        "wcat": np.concatenate([whi, wlo], axis=1),
        "nasq": np.stack([(-a) * sq for a in alphas]).astype(np.float32),
        "rbf": np.stack(
            [w * np.exp((-a) * sq.astype(np.float64)) for a, w in zip(alphas, weights)]
        ).astype(bf),
    }
    if need_b:
        m["brow"] = b_proj
    if need_gamma:
        m["grow"] = ln_gamma
    if need_beta:
        m["berow"] = ln_beta
    return m


def _specialize(inputs):
    x = np.asarray(inputs["x"], dtype=np.float32)
    log_sigmas = np.asarray(inputs["log_sigmas"], dtype=np.float32)
    W_proj = np.asarray(inputs["W_proj"], dtype=np.float32)
    b_proj = np.ascontiguousarray(np.asarray(inputs["b_proj"], dtype=np.float32))
    ln_gamma = np.ascontiguousarray(np.asarray(inputs["ln_gamma"], dtype=np.float32))
    ln_beta = np.ascontiguousarray(np.asarray(inputs["ln_beta"], dtype=np.float32))

    sigmas = np.exp(log_sigmas)
    denoms = (np.float32(2.0) * sigmas * sigmas + np.float32(1e-6)).astype(np.float32)
    uniq, counts = np.unique(denoms, return_counts=True)
    alphas = (np.float32(1.0) / uniq).astype(np.float32)
    weights = counts.astype(np.float32) / np.float32(H)
    two_alphas = tuple(float(2.0 * a) for a in alphas)

    need_b = bool(np.any(b_proj != 0))
    need_gamma = not bool(np.all(ln_gamma == 1))
    need_beta = bool(np.any(ln_beta != 0))
    return (x, W_proj, b_proj, ln_gamma, ln_beta, alphas, weights, two_alphas,
            need_b, need_gamma, need_beta)


def kernel(**inputs):
    from concourse import bass_utils

    (x, W_proj, b_proj, ln_gamma, ln_beta, alphas, weights, two_alphas,
     need_b, need_gamma, need_beta) = _specialize(inputs)

    assert x.shape == (B, N, D), x.shape

    key = (len(alphas), two_alphas, tuple(float(v) for v in weights),
           need_b, need_gamma, need_beta)
    if key not in _PROGRAM_CACHE:
        _PROGRAM_CACHE[key] = _build_program(
            len(alphas), two_alphas, need_b, need_gamma, need_beta
        )
    nc = _PROGRAM_CACHE[key]

    W_T = np.ascontiguousarray(W_proj.T)
    in_maps = [
        _prepare_core_inputs(x[k], alphas, weights, W_T, b_proj, ln_gamma, ln_beta,
                             need_b, need_gamma, need_beta)
        for k in range(B)
    ]

    res = bass_utils.run_bass_kernel_spmd(nc, in_maps, core_ids=list(range(B)))
    out = np.stack([res.results[k]["out"] for k in range(B)])
    return out.astype(np.float32)


if __name__ == "__main__":
    import reference as R

    inp = R.setup_inputs()
    got = kernel(**{k: np.asarray(v) for k, v in inp.items()})
    print("out shape", got.shape, got.dtype)
